# revision 26
# baseline (speedup 1.0000x reference)
"""DualGAT (2-hop, 2-graph GAT + gated fuse + MLP) on 8 Trainium2 NeuronCores.

Math per GAT layer/head (z[v,u] = s_v + t_u):
    exp(LeakyRelu(z, 0.2)) = p_v q_u max(exp(0.8 z), 1)      (exact)
    exp(0.8 z) = R_v r_u                                      (separable)
with p = exp(0.2 s), q = exp(0.2 t), R = exp(0.8 s), r = exp(0.8 t).
The p_v factor is common to numerator and denominator and cancels, so
    H[v] = (sum_u adj * w * q_u [Wh_u|1]) / den,  w = max(R_v r_u, 1)
One 4x-mode DVE tensor_scalar per (chunk, head) builds w = (R_b * r) max 1,
one 2x TT per 8-chunk group applies the adjacency mask in place, and a
single M=32 bf16 matmul per (chunk, head) accumulates num|den into PSUM.

Sharding: v (attention rows) split 8 ways, 384 rows/core; u (neighbors) full.
Feature tensors use padded 4x32 head blocks (col 16 = softmax denominator,
~1.0 junk after normalize); downstream weight rows are zero-padded there.
"""

import sys
import numpy as np

for _p in ("/opt/trn_rl_repo",):
    if _p not in sys.path:
        sys.path.insert(0, _p)

import ml_dtypes

N = 3072
IN_DIM = 32
HID = 64
HEADS = 4
HD = 16
NCORES = 8
VL = N // NCORES          # 384
P = 128
UC = N // P               # 24
FP = 128                  # padded feature rows: 4 heads x 32
MH = HID // 2
KROWS = [IN_DIM, FP]
BLK = 32
GOFF = [0, 72]
SOFF = [64, 136]
TOFF = [68, 140]
CG = 8                    # chunks per mask group

NO_COLLECTIVE = False

_CACHE = {}


def _build():
    import concourse.bacc as bacc
    import concourse.mybir as mybir
    from concourse.tile import TileContext

    dt = mybir.dt
    op = mybir.AluOpType
    AF = mybir.ActivationFunctionType
    bf = dt.bfloat16

    nc = bacc.Bacc("TRN2", target_bir_lowering=False, debug=False,
                   num_devices=NCORES)

    def dram_in(name, shape, dtype=bf):
        return nc.dram_tensor(name, list(shape), dtype, kind="ExternalInput")

    xT_d = dram_in("xT", (IN_DIM, N))
    xOwn_d = dram_in("xOwnT", (IN_DIM, VL))
    adj_d = [dram_in("adjTB_i", (P, UC * VL)),
             dram_in("adjTB_c", (P, UC * VL))]
    WST_d = [dram_in(f"WST{l}", (KROWS[l], 144)) for l in range(2)]
    qg_d = [dram_in(f"qg{l}", (FP, 2)) for l in range(2)]
    mw1_d = dram_in("mw1", (FP, MH))
    mb1_d = dram_in("mb1", (MH, 1), dt.float32)
    mw2_d = dram_in("mw2", (MH, 1))
    mb2_d = dram_in("mb2", (1, 1), dt.float32)
    out_d = nc.dram_tensor("out", [1, VL], dt.float32, kind="ExternalOutput")

    # inline consts
    sel4_np = np.zeros((HEADS, HEADS * P), dtype=np.float32)
    for h in range(HEADS):
        sel4_np[h, P * h:P * (h + 1)] = 1.0
    sel4_d = nc.inline_tensor(sel4_np.astype(ml_dtypes.bfloat16), name="sel4")
    e17_np = np.zeros((HEADS, FP), dtype=np.float32)
    for h in range(HEADS):
        e17_np[h, BLK * h:BLK * (h + 1)] = 1.0
    e17_d = nc.inline_tensor(e17_np.astype(ml_dtypes.bfloat16), name="e17")
    one1_d = nc.inline_tensor(np.ones((1, P), dtype=np.float32)
                              .astype(ml_dtypes.bfloat16), name="one1")

    def sb(name, shape, dtype=dt.float32):
        return nc.alloc_sbuf_tensor(name, list(shape), dtype).ap()

    xT = sb("s_xT", (IN_DIM, N), bf)
    XOWN = sb("s_xOwn", (IN_DIM, VL), bf)
    adjTB = [sb(f"s_adjTB{g}", (P, UC * VL), bf) for g in range(2)]
    WSTs = [sb(f"s_WST{l}", (KROWS[l], 144), bf) for l in range(2)]
    WH = sb("s_WH", (P, UC * 144), bf)
    H1T = sb("s_H1T", (FP, N), bf)
    GW = [sb(f"s_GW{g}", (P, UC * HEADS * BLK), bf) for g in range(2)]
    RB4 = [sb(f"s_RB4_{g}", (P, HEADS * VL), bf) for g in range(2)]
    RCOL = [sb(f"s_RCOL{g}", (P, UC * HEADS)) for g in range(2)]
    QCOL = [sb(f"s_QCOL{g}", (P, UC * HEADS)) for g in range(2)]
    RRB = [sb(f"s_RRB{g}", (HEADS, VL), bf) for g in range(2)]
    HE = [sb(f"s_HE{g}", (FP, VL), bf) for g in range(2)]
    HF1 = sb("s_HF1", (FP, VL), bf)
    HF2 = sb("s_HF2", (FP, VL), bf)
    SEL4s = sb("s_sel4", (HEADS, HEADS * P), bf)
    E17s = sb("s_e17", (HEADS, FP), bf)
    ONE1s = sb("s_one1", (1, P), bf)
    QGs = [sb(f"s_qg{l}", (FP, 2), bf) for l in range(2)]
    MW1 = sb("s_mw1", (FP, MH), bf)
    MB1 = sb("s_mb1", (MH, 1))
    MW2 = sb("s_mw2", (MH, 1), bf)
    MB2 = sb("s_mb2", (1, 1))

    WH_v = WH.rearrange("p (k c) -> p k c", c=144)
    adj_v = [a.rearrange("p (k v) -> p k v", v=VL) for a in adjTB]
    GW_v = [g.rearrange("p (k h c) -> p k h c", h=HEADS, c=BLK) for g in GW]
    RB4_v = [r.rearrange("p (h v) -> p h v", h=HEADS) for r in RB4]
    QCOL_v = [q.rearrange("p (k h) -> p k h", h=HEADS) for q in QCOL]
    RCOL_v = [r.rearrange("p (k h) -> p k h", h=HEADS) for r in RCOL]

    with TileContext(nc) as tc:
        with tc.tile_pool(name="w8p", bufs=3) as wp, \
             tc.tile_pool(name="work", bufs=5) as work, \
             tc.tile_pool(name="small", bufs=6) as smp, \
             tc.tile_pool(name="ps_w", bufs=2, space="PSUM") as ps_w, \
             tc.tile_pool(name="ps_agg", bufs=2, space="PSUM") as ps_agg, \
             tc.tile_pool(name="ps_m", bufs=3, space="PSUM") as ps_m, \
             tc.tile_pool(name="dram", bufs=1, space="DRAM") as drp:

            # ---------- loads: adjacency first (largest, needed by hop-1) ----
            nc.sync.dma_start(out=xT[:], in_=xT_d.ap())
            nc.sync.dma_start(out=XOWN[:], in_=xOwn_d.ap())
            for l in range(2):
                nc.sync.dma_start(out=WSTs[l][:], in_=WST_d[l].ap())
            nc.sync.dma_start(out=SEL4s[:], in_=sel4_d.ap())
            adjg_d = [a.ap().rearrange("p (k v) -> p k v", v=VL) for a in adj_d]
            for g in range(2):
                nc.sync.dma_start(out=adj_v[g][:, 0:CG, :],
                                  in_=adjg_d[g][:, 0:CG, :])
            nc.sync.dma_start(out=E17s[:], in_=e17_d.ap())
            nc.sync.dma_start(out=ONE1s[:], in_=one1_d.ap())
            for l in range(2):
                nc.sync.dma_start(out=QGs[l][:], in_=qg_d[l].ap())
            for i in range(1, UC // CG):
                sl = slice(i * CG, (i + 1) * CG)
                for g in range(2):
                    nc.sync.dma_start(out=adj_v[g][:, sl, :],
                                      in_=adjg_d[g][:, sl, :])
            nc.sync.dma_start(out=MW1[:], in_=mw1_d.ap())
            nc.sync.dma_start(out=MB1[:], in_=mb1_d.ap())
            nc.sync.dma_start(out=MW2[:], in_=mw2_d.ap())
            nc.sync.dma_start(out=MB2[:], in_=mb2_d.ap())
            for g in range(2):
                nc.gpsimd.memset(GW_v[g][:, :, :, HD + 1:BLK], 0.0)

            def gat_prologueA(l, hown):
                """Own-slice path: R = exp(0.8 s) broadcast into RB4 (bf16).
                Depends only on hown + weights, not on the all-gathered HT."""
                krows = KROWS[l]
                wst = WSTs[l]
                for g in range(2):
                    st_ps = ps_m.tile([8, VL], dt.float32, tag="m")
                    nc.tensor.matmul(st_ps[:], wst[0:krows, SOFF[g]:SOFF[g] + 8],
                                     hown[:], start=True, stop=True)
                    nc.scalar.activation(RRB[g][:], st_ps[0:HEADS, :], AF.Exp,
                                         scale=0.8)
                    for h in range(HEADS):
                        rps = ps_m.tile([P, VL], dt.float32, tag="m")
                        nc.tensor.matmul(rps[:], SEL4s[:, P * h:P * (h + 1)],
                                         RRB[g][:], start=True, stop=True)
                        nc.vector.tensor_copy(out=RB4_v[g][:, h, :],
                                              in_=rps[:])

            def gat_main(l, HT, hf_out):
                krows = KROWS[l]
                wst = WSTs[l]
                NG = UC // CG

                def stwh_group(i):
                    sl = slice(i * CG, (i + 1) * CG)
                    for k in range(i * CG, (i + 1) * CG):
                        stwh = ps_w.tile([P, 144], dt.float32, tag="stwh")
                        nc.tensor.matmul(stwh[:], HT[:, P * k:P * (k + 1)],
                                         wst[0:krows, :], start=True, stop=True)
                        nc.scalar.copy(WH_v[:, k, :], stwh[:])
                    for g in range(2):
                        tcols = WH_v[:, sl, TOFF[g]:TOFF[g] + HEADS]
                        nc.scalar.activation(QCOL_v[g][:, sl, :], tcols,
                                             AF.Exp, scale=0.2)
                        nc.scalar.activation(RCOL_v[g][:, sl, :], tcols,
                                             AF.Exp, scale=0.8)
                        nc.gpsimd.tensor_tensor(
                            out=GW_v[g][:, sl, :, 0:HD],
                            in0=WH_v[:, sl, GOFF[g]:GOFF[g] + HID].rearrange(
                                "p k (h d) -> p k h d", d=HD),
                            in1=QCOL_v[g][:, sl, :, None].to_broadcast(
                                (P, CG, HEADS, HD)),
                            op=op.mult)
                        nc.gpsimd.tensor_copy(out=GW_v[g][:, sl, :, HD],
                                              in_=QCOL_v[g][:, sl, :])

                def core_group(g, k0, cg, P_agg, npool):
                    """cg chunks from k0; first npool chunks' weight TSPs
                    run on Pool to offload DVE."""
                    w8f = wp.tile([P, CG, HEADS, VL], bf, tag="w8")
                    w8 = w8f[:, 0:cg]
                    for c in range(cg):
                        k = k0 + c
                        eng = nc.gpsimd if c < npool else nc.vector
                        for h in range(HEADS):
                            eng.tensor_scalar(
                                w8[:, c, h, :], RB4_v[g][:, h, :],
                                RCOL[g][:, k * HEADS + h:k * HEADS + h + 1],
                                1.0, op.mult, op.max)
                    nc.vector.tensor_tensor(
                        out=w8[:], in0=w8[:],
                        in1=adj_v[g][:, k0:k0 + cg, :][:, :, None, :]
                            .to_broadcast((P, cg, HEADS, VL)),
                        op=op.mult)
                    for c in range(cg):
                        k = k0 + c
                        for h in range(HEADS):
                            nc.tensor.matmul(
                                P_agg[BLK * h:BLK * h + BLK, :],
                                GW_v[g][:, k, h, :], w8[:, c, h, :],
                                start=(k == 0), stop=(k == UC - 1),
                                tile_position=(0, BLK * h))

                def epi_header(g, P_agg):
                    """Act copy + den gather DMA — no DVE ops."""
                    ncs = work.tile([FP, VL], bf, tag="w", name=f"ncs{g}")
                    nc.scalar.copy(ncs[:], P_agg[:])
                    den4 = smp.tile([HEADS, VL], bf, tag="s", name=f"den{g}")
                    nc.gpsimd.dma_start(out=den4[:], in_=ncs[HD::BLK, :])
                    return ncs, den4

                def epi_tail(g, ncs, den4):
                    rdf = smp.tile([HEADS, VL], bf, tag="s")
                    with nc.allow_low_precision(reason="den recip, 0.4% ok"):
                        nc.vector.reciprocal(rdf[:], den4[:])
                    rb_ps = ps_m.tile([FP, VL], dt.float32, tag="m")
                    nc.tensor.matmul(rb_ps[:], E17s[:], rdf[:],
                                     start=True, stop=True)
                    hgx = work.tile([FP, VL], dt.float32, tag="w")
                    nc.vector.tensor_tensor(out=hgx[:], in0=ncs[:],
                                            in1=rb_ps[:], op=op.mult)
                    r0 = work.tile([FP, VL], dt.float32, tag="w")
                    nc.scalar.activation(r0[:], hgx[:], AF.Relu)
                    rn = work.tile([FP, VL], dt.float32, tag="w")
                    nc.scalar.activation(rn[:], hgx[:], AF.Relu, scale=-1.0)
                    em = work.tile([FP, VL], dt.float32, tag="w")
                    nc.scalar.activation(em[:], rn[:], AF.Exp, scale=-1.0)
                    nc.vector.scalar_tensor_tensor(
                        out=HE[g][:], in0=r0[:], scalar=-1.0, in1=em[:],
                        op0=op.add, op1=op.add)
                    # this graph's fuse leg (PE+Act only)
                    ai_ps = ps_m.tile([1, VL], dt.float32, tag="m")
                    nc.tensor.matmul(ai_ps[:], QGs[l][:, g:g + 1], HE[g][:],
                                     start=True, stop=True)
                    e = smp.tile([1, VL], bf, tag="s")
                    nc.scalar.activation(e[:], ai_ps[:], AF.Exp)
                    return e

                # software-pipelined: g0 core first, its epilogue DVE work
                # threaded between g1 core groups; last groups small so the
                # final accumulation (and epilogue) finishes early
                CGS = [(0, 8, 0), (8, 8, 0), (16, 6, 0), (22, 2, 0)]
                stwh_group(0)
                P_aggs = [ps_agg.tile([FP, VL], dt.float32, tag="agg",
                                      name=f"pagg{g}")
                          for g in range(2)]
                for j, (k0, cg, npool) in enumerate(CGS):
                    if j + 1 < NG:
                        stwh_group(j + 1)
                    core_group(0, k0, cg, P_aggs[0], npool)
                ncs0, den0 = epi_header(0, P_aggs[0])
                core_group(1, *CGS[0][:2], P_aggs[1], CGS[0][2])
                core_group(1, *CGS[1][:2], P_aggs[1], CGS[1][2])
                e0 = epi_tail(0, ncs0, den0)
                core_group(1, *CGS[2][:2], P_aggs[1], CGS[2][2])
                core_group(1, *CGS[3][:2], P_aggs[1], CGS[3][2])
                ncs1, den1 = epi_header(1, P_aggs[1])
                e1 = epi_tail(1, ncs1, den1)

                # gated fuse
                ei = [e0, e1]
                dsum = smp.tile([1, VL], dt.float32, tag="s")
                nc.vector.tensor_tensor(out=dsum[:], in0=ei[0][:],
                                        in1=ei[1][:], op=op.add)
                rds = smp.tile([1, VL], dt.float32, tag="s")
                nc.vector.reciprocal(rds[:], dsum[:])
                b0 = smp.tile([1, VL], bf, tag="s")
                nc.vector.tensor_tensor(out=b0[:], in0=ei[0][:], in1=rds[:],
                                        op=op.mult)
                bib_ps = ps_m.tile([FP, VL], dt.float32, tag="m")
                nc.tensor.matmul(bib_ps[:], ONE1s[:], b0[:],
                                 start=True, stop=True)
                dd = work.tile([FP, VL], bf, tag="w")
                nc.vector.tensor_tensor(out=dd[:], in0=HE[0][:], in1=HE[1][:],
                                        op=op.subtract)
                bd = work.tile([FP, VL], bf, tag="w")
                nc.vector.tensor_tensor(out=bd[:], in0=dd[:], in1=bib_ps[:],
                                        op=op.mult)
                if hf_out is None:
                    return bd          # caller folds HE[1] + bd linearly
                nc.vector.tensor_tensor(out=hf_out[:], in0=HE[1][:],
                                        in1=bd[:], op=op.add)

            # ---------------- hop 1 ----------------
            gat_prologueA(0, XOWN)
            gat_main(0, xT, HF1)

            # all-gather H1 (feature-major, bf16); hop-2 own-slice prologue
            # runs under the collective (depends only on HF1).
            ag_in = drp.tile([FP, VL], bf)
            ag_out = drp.tile([NCORES, FP, VL], bf)
            nc.sync.dma_start(out=ag_in[:], in_=HF1[:])
            agv = ag_out.opt().rearrange("c (f v) -> c f v", v=VL)
            h1v = H1T.rearrange("f (c v) -> f c v", v=VL)
            if NO_COLLECTIVE:
                for c in range(NCORES):
                    nc.sync.dma_start(out=agv[c], in_=ag_in[:])
                    nc.sync.dma_start(out=h1v[:, c, :], in_=agv[c])
            else:
                nc.gpsimd.collective_compute(
                    "AllGather", op.bypass,
                    replica_groups=[list(range(NCORES))],
                    ins=[ag_in.opt()], outs=[ag_out.opt()])
                for c in range(NCORES):
                    nc.sync.dma_start(out=h1v[:, c, :], in_=agv[c])
            gat_prologueA(1, HF1)

            # ---------------- hop 2 + MLP head ----------------
            # H2 = HE[1] + bd is folded into the first MLP matmul (linear)
            bd2 = gat_main(1, H1T, None)
            h_ps = ps_m.tile([MH, VL], dt.float32, tag="m")
            nc.tensor.matmul(h_ps[:], MW1[:], HE[1][:], start=True, stop=False)
            nc.tensor.matmul(h_ps[:], MW1[:], bd2[:], start=False, stop=True)
            hd = smp.tile([MH, VL], bf, tag="s")
            nc.vector.tensor_scalar(hd[:], h_ps[:], MB1[:], 0.0,
                                    op.add, op.max)
            o_ps = ps_m.tile([1, VL], dt.float32, tag="m")
            nc.tensor.matmul(o_ps[:], MW2[:], hd[:], start=True, stop=True)
            osb = smp.tile([1, VL], dt.float32, tag="s")
            nc.vector.tensor_scalar(osb[:], o_ps[:], MB2[:], None,
                                    op.add, op.bypass)
            nc.sync.dma_start(out=out_d.ap(), in_=osb[:])

    nc.compile()
    return nc


def _pad_rows(w):
    out = np.zeros((FP,) + w.shape[1:], dtype=np.float32)
    for h in range(HEADS):
        out[BLK * h:BLK * h + HD] = w[HD * h:HD * h + HD]
    return out


def _ahat(a):
    A = np.zeros((HID, 2 * HEADS), dtype=np.float32)
    for h in range(HEADS):
        A[HD * h:HD * h + HD, h] = a[h, :HD]
        A[HD * h:HD * h + HD, HEADS + h] = a[h, HD:]
    return A


def _prep_adj(adj, c):
    """(N,N) int -> per-core (P, UC*VL) bf16 {0,1} chunk layout of adjT."""
    sl = adj[c * VL:(c + 1) * VL, :].T.astype(np.float32)       # (N, VL)
    sl = sl.reshape(UC, P, VL).transpose(1, 0, 2).reshape(P, UC * VL)
    return np.ascontiguousarray(sl).astype(ml_dtypes.bfloat16)


def _bf(x):
    return np.ascontiguousarray(x).astype(ml_dtypes.bfloat16)


def kernel(**inputs):
    from concourse.bass_utils import run_bass_kernel_spmd

    if "nc" not in _CACHE:
        _CACHE["nc"] = _build()
    nc = _CACHE["nc"]

    f32 = np.float32
    x = np.asarray(inputs["x"], f32)
    adj = [np.asarray(inputs["adj_ind"]), np.asarray(inputs["adj_cor"])]
    W1 = [np.asarray(inputs["W1i"], f32), np.asarray(inputs["W1c"], f32)]
    W2 = [np.asarray(inputs["W2i"], f32), np.asarray(inputs["W2c"], f32)]
    A1 = [np.asarray(inputs["a1i"], f32), np.asarray(inputs["a1c"], f32)]
    A2 = [np.asarray(inputs["a2i"], f32), np.asarray(inputs["a2c"], f32)]
    q1 = [np.asarray(inputs["q1i"], f32), np.asarray(inputs["q1c"], f32)]
    q2 = [np.asarray(inputs["q2i"], f32), np.asarray(inputs["q2c"], f32)]

    common = {"xT": _bf(x.T)}
    for l, (Ws, As) in enumerate(((W1, A1), (W2, A2))):
        blocks = []
        for g in range(2):
            W = Ws[g] if l == 0 else _pad_rows(Ws[g])
            WA = W @ _ahat(As[g])
            blocks.append(np.hstack([W, WA]))
        common[f"WST{l}"] = _bf(np.hstack(blocks))
    for l, qs in enumerate((q1, q2)):
        common[f"qg{l}"] = _bf(
            np.stack([_pad_rows(qs[0][:, None])[:, 0],
                      _pad_rows(qs[1][:, None])[:, 0]], axis=1))
    common["mw1"] = _bf(_pad_rows(np.asarray(inputs["mlp_w1"], f32)))
    common["mb1"] = np.ascontiguousarray(
        np.asarray(inputs["mlp_b1"], f32)[:, None])
    common["mw2"] = _bf(np.asarray(inputs["mlp_w2"], f32))
    common["mb2"] = np.asarray(inputs["mlp_b2"], f32).reshape(1, 1)

    in_maps = []
    for c in range(NCORES):
        m = dict(common)
        m["xOwnT"] = _bf(x[c * VL:(c + 1) * VL, :].T)
        m["adjTB_i"] = _prep_adj(adj[0], c)
        m["adjTB_c"] = _prep_adj(adj[1], c)
        in_maps.append(m)

    res = run_bass_kernel_spmd(nc, in_maps, core_ids=list(range(NCORES)))
    out = np.concatenate([r["out"][0] for r in res.results])[:, None]
    return out.astype(np.float32)


if __name__ == "__main__":
    _CACHE["nc"] = _build()
    print("build ok")


# revision 29
# speedup vs baseline: 1.0682x; 1.0682x over previous
"""DualGAT (2-hop, 2-graph GAT + gated fuse + MLP) on 8 Trainium2 NeuronCores.

Math per GAT layer/head (z[v,u] = s_v + t_u):
    exp(LeakyRelu(z, 0.2)) = p_v q_u max(exp(0.8 z), 1)      (exact)
    exp(0.8 z) = R_v r_u                                      (separable)
with p = exp(0.2 s), q = exp(0.2 t), R = exp(0.8 s), r = exp(0.8 t).
The p_v factor is common to numerator and denominator and cancels, so
    H[v] = (sum_u adj * w * q_u [Wh_u|1]) / den,  w = max(R_v r_u, 1)
One 4x-mode DVE tensor_scalar per (chunk, head) builds w = (R_b * r) max 1,
one 2x TT per 8-chunk group applies the adjacency mask in place, and a
single M=32 bf16 matmul per (chunk, head) accumulates num|den into PSUM.

Sharding: v (attention rows) split 8 ways, 384 rows/core; u (neighbors) full.
Feature tensors use padded 4x32 head blocks (col 16 = softmax denominator,
~1.0 junk after normalize); downstream weight rows are zero-padded there.
"""

import sys
import numpy as np

for _p in ("/opt/trn_rl_repo",):
    if _p not in sys.path:
        sys.path.insert(0, _p)

import ml_dtypes

N = 3072
IN_DIM = 32
HID = 64
HEADS = 4
HD = 16
NCORES = 8
VL = N // NCORES          # 384
P = 128
UC = N // P               # 24
FP = 128                  # padded feature rows: 4 heads x 32
MH = HID // 2
KROWS = [IN_DIM, FP]
BLK = 32
GOFF = [0, 72]
SOFF = [64, 136]
TOFF = [68, 140]
CG = 8                    # chunks per mask group

NO_COLLECTIVE = False

_CACHE = {}


def _build():
    import concourse.bacc as bacc
    import concourse.mybir as mybir
    from concourse.tile import TileContext

    dt = mybir.dt
    op = mybir.AluOpType
    AF = mybir.ActivationFunctionType
    bf = dt.bfloat16

    nc = bacc.Bacc("TRN2", target_bir_lowering=False, debug=False,
                   num_devices=NCORES)

    def dram_in(name, shape, dtype=bf):
        return nc.dram_tensor(name, list(shape), dtype, kind="ExternalInput")

    xT_d = dram_in("xT", (IN_DIM, N))
    xOwn_d = dram_in("xOwnT", (IN_DIM, VL))
    adj_d = [dram_in("adjTB_i", (P, UC * VL)),
             dram_in("adjTB_c", (P, UC * VL))]
    WST_d = [dram_in(f"WST{l}", (KROWS[l], 144)) for l in range(2)]
    qg_d = [dram_in(f"qg{l}", (FP, 2)) for l in range(2)]
    mw1_d = dram_in("mw1", (FP, MH))
    mb1_d = dram_in("mb1", (MH, 1), dt.float32)
    mw2_d = dram_in("mw2", (MH, 1))
    mb2_d = dram_in("mb2", (1, 1), dt.float32)
    out_d = nc.dram_tensor("out", [1, VL], dt.float32, kind="ExternalOutput")

    # inline consts
    sel4_np = np.zeros((HEADS, HEADS * P), dtype=np.float32)
    for h in range(HEADS):
        sel4_np[h, P * h:P * (h + 1)] = 1.0
    sel4_d = nc.inline_tensor(sel4_np.astype(ml_dtypes.bfloat16), name="sel4")
    e17_np = np.zeros((HEADS, FP), dtype=np.float32)
    for h in range(HEADS):
        e17_np[h, BLK * h:BLK * (h + 1)] = 1.0
    e17_d = nc.inline_tensor(e17_np.astype(ml_dtypes.bfloat16), name="e17")
    one1_d = nc.inline_tensor(np.ones((1, P), dtype=np.float32)
                              .astype(ml_dtypes.bfloat16), name="one1")

    def sb(name, shape, dtype=dt.float32):
        return nc.alloc_sbuf_tensor(name, list(shape), dtype).ap()

    xT = sb("s_xT", (IN_DIM, N), bf)
    XOWN = sb("s_xOwn", (IN_DIM, VL), bf)
    adjTB = [sb(f"s_adjTB{g}", (P, UC * VL), bf) for g in range(2)]
    WSTs = [sb(f"s_WST{l}", (KROWS[l], 144), bf) for l in range(2)]
    WH = sb("s_WH", (P, UC * 144), bf)
    H1T = sb("s_H1T", (FP, N), bf)
    GW = [sb(f"s_GW{g}", (P, UC * HEADS * BLK), bf) for g in range(2)]
    RB4 = [sb(f"s_RB4_{g}", (P, HEADS * VL), bf) for g in range(2)]
    RCOL = [sb(f"s_RCOL{g}", (P, UC * HEADS)) for g in range(2)]
    QCOL = [sb(f"s_QCOL{g}", (P, UC * HEADS)) for g in range(2)]
    RRB = [sb(f"s_RRB{g}", (HEADS, VL), bf) for g in range(2)]
    HE = [sb(f"s_HE{g}", (FP, VL), bf) for g in range(2)]
    HF1 = sb("s_HF1", (FP, VL), bf)
    HF2 = sb("s_HF2", (FP, VL), bf)
    SEL4s = sb("s_sel4", (HEADS, HEADS * P), bf)
    E17s = sb("s_e17", (HEADS, FP), bf)
    ONE1s = sb("s_one1", (1, P), bf)
    QGs = [sb(f"s_qg{l}", (FP, 2), bf) for l in range(2)]
    MW1 = sb("s_mw1", (FP, MH), bf)
    MB1 = sb("s_mb1", (MH, 1))
    MW2 = sb("s_mw2", (MH, 1), bf)
    MB2 = sb("s_mb2", (1, 1))

    WH_v = WH.rearrange("p (k c) -> p k c", c=144)
    adj_v = [a.rearrange("p (k v) -> p k v", v=VL) for a in adjTB]
    GW_v = [g.rearrange("p (k h c) -> p k h c", h=HEADS, c=BLK) for g in GW]
    RB4_v = [r.rearrange("p (h v) -> p h v", h=HEADS) for r in RB4]
    QCOL_v = [q.rearrange("p (k h) -> p k h", h=HEADS) for q in QCOL]
    RCOL_v = [r.rearrange("p (k h) -> p k h", h=HEADS) for r in RCOL]

    with TileContext(nc) as tc:
        with tc.tile_pool(name="w8p", bufs=3) as wp, \
             tc.tile_pool(name="work", bufs=5) as work, \
             tc.tile_pool(name="small", bufs=6) as smp, \
             tc.tile_pool(name="ps_w", bufs=2, space="PSUM") as ps_w, \
             tc.tile_pool(name="ps_agg", bufs=2, space="PSUM") as ps_agg, \
             tc.tile_pool(name="ps_m", bufs=3, space="PSUM") as ps_m, \
             tc.tile_pool(name="dram", bufs=1, space="DRAM") as drp:

            # ---------- loads: adjacency first (largest, needed by hop-1) ----
            nc.sync.dma_start(out=xT[:], in_=xT_d.ap())
            nc.sync.dma_start(out=XOWN[:], in_=xOwn_d.ap())
            for l in range(2):
                nc.sync.dma_start(out=WSTs[l][:], in_=WST_d[l].ap())
            nc.sync.dma_start(out=SEL4s[:], in_=sel4_d.ap())
            adjg_d = [a.ap().rearrange("p (k v) -> p k v", v=VL) for a in adj_d]
            for g in range(2):
                nc.sync.dma_start(out=adj_v[g][:, 0:CG, :],
                                  in_=adjg_d[g][:, 0:CG, :])
            nc.sync.dma_start(out=E17s[:], in_=e17_d.ap())
            nc.sync.dma_start(out=ONE1s[:], in_=one1_d.ap())
            for l in range(2):
                nc.sync.dma_start(out=QGs[l][:], in_=qg_d[l].ap())
            for i in range(1, UC // CG):
                sl = slice(i * CG, (i + 1) * CG)
                for g in range(2):
                    nc.sync.dma_start(out=adj_v[g][:, sl, :],
                                      in_=adjg_d[g][:, sl, :])
            nc.sync.dma_start(out=MW1[:], in_=mw1_d.ap())
            nc.sync.dma_start(out=MB1[:], in_=mb1_d.ap())
            nc.sync.dma_start(out=MW2[:], in_=mw2_d.ap())
            nc.sync.dma_start(out=MB2[:], in_=mb2_d.ap())
            for g in range(2):
                nc.gpsimd.memset(GW_v[g][:, :, :, HD + 1:BLK], 0.0)

            def gat_prologueA(l, hown):
                """Own-slice path: R = exp(0.8 s) broadcast into RB4 (bf16).
                Depends only on hown + weights, not on the all-gathered HT."""
                krows = KROWS[l]
                wst = WSTs[l]
                for g in range(2):
                    st_ps = ps_m.tile([8, VL], dt.float32, tag="m")
                    nc.tensor.matmul(st_ps[:], wst[0:krows, SOFF[g]:SOFF[g] + 8],
                                     hown[:], start=True, stop=True)
                    nc.scalar.activation(RRB[g][:], st_ps[0:HEADS, :], AF.Exp,
                                         scale=0.8)
                    for h in range(HEADS):
                        rps = ps_m.tile([P, VL], dt.float32, tag="m")
                        nc.tensor.matmul(rps[:], SEL4s[:, P * h:P * (h + 1)],
                                         RRB[g][:], start=True, stop=True)
                        nc.vector.tensor_copy(out=RB4_v[g][:, h, :],
                                              in_=rps[:])

            def gat_main(l, HT, hf_out):
                krows = KROWS[l]
                wst = WSTs[l]
                NG = UC // CG

                def stwh_group(i):
                    sl = slice(i * CG, (i + 1) * CG)
                    for k in range(i * CG, (i + 1) * CG):
                        stwh = ps_w.tile([P, 144], dt.float32, tag="stwh")
                        nc.tensor.matmul(stwh[:], HT[:, P * k:P * (k + 1)],
                                         wst[0:krows, :], start=True, stop=True)
                        nc.scalar.copy(WH_v[:, k, :], stwh[:])
                    for g in range(2):
                        tcols = WH_v[:, sl, TOFF[g]:TOFF[g] + HEADS]
                        nc.scalar.activation(QCOL_v[g][:, sl, :], tcols,
                                             AF.Exp, scale=0.2)
                        nc.scalar.activation(RCOL_v[g][:, sl, :], tcols,
                                             AF.Exp, scale=0.8)
                        nc.gpsimd.tensor_tensor(
                            out=GW_v[g][:, sl, :, 0:HD],
                            in0=WH_v[:, sl, GOFF[g]:GOFF[g] + HID].rearrange(
                                "p k (h d) -> p k h d", d=HD),
                            in1=QCOL_v[g][:, sl, :, None].to_broadcast(
                                (P, CG, HEADS, HD)),
                            op=op.mult)
                        nc.gpsimd.tensor_copy(out=GW_v[g][:, sl, :, HD],
                                              in_=QCOL_v[g][:, sl, :])

                def core_group(g, k0, cg, P_agg, npool):
                    """cg chunks from k0; first npool chunks' weight TSPs
                    run on Pool to offload DVE."""
                    w8f = wp.tile([P, CG, HEADS, VL], bf, tag="w8")
                    w8 = w8f[:, 0:cg]
                    for c in range(cg):
                        k = k0 + c
                        eng = nc.gpsimd if c < npool else nc.vector
                        for h in range(HEADS):
                            eng.tensor_scalar(
                                w8[:, c, h, :], RB4_v[g][:, h, :],
                                RCOL[g][:, k * HEADS + h:k * HEADS + h + 1],
                                1.0, op.mult, op.max)
                    nc.vector.tensor_tensor(
                        out=w8[:], in0=w8[:],
                        in1=adj_v[g][:, k0:k0 + cg, :][:, :, None, :]
                            .to_broadcast((P, cg, HEADS, VL)),
                        op=op.mult)
                    for c in range(cg):
                        k = k0 + c
                        for h in range(HEADS):
                            nc.tensor.matmul(
                                P_agg[BLK * h:BLK * h + BLK, :],
                                GW_v[g][:, k, h, :], w8[:, c, h, :],
                                start=(k == 0), stop=(k == UC - 1),
                                tile_position=(0, BLK * h))

                def epi_header(g, P_agg):
                    """Act copy + den gather DMA — no DVE ops."""
                    ncs = work.tile([FP, VL], bf, tag="w", name=f"ncs{g}")
                    nc.scalar.copy(ncs[:], P_agg[:])
                    den4 = smp.tile([HEADS, VL], bf, tag="s", name=f"den{g}")
                    nc.sync.dma_start(out=den4[:], in_=ncs[HD::BLK, :])
                    return ncs, den4

                def epi_tail(g, ncs, den4):
                    rdf = smp.tile([HEADS, VL], bf, tag="s")
                    with nc.allow_low_precision(reason="den recip, 0.4% ok"):
                        nc.vector.reciprocal(rdf[:], den4[:])
                    rb_ps = ps_m.tile([FP, VL], dt.float32, tag="m")
                    nc.tensor.matmul(rb_ps[:], E17s[:], rdf[:],
                                     start=True, stop=True)
                    hgx = work.tile([FP, VL], dt.float32, tag="w")
                    nc.vector.tensor_tensor(out=hgx[:], in0=ncs[:],
                                            in1=rb_ps[:], op=op.mult)
                    r0 = work.tile([FP, VL], dt.float32, tag="w")
                    nc.scalar.activation(r0[:], hgx[:], AF.Relu)
                    rn = work.tile([FP, VL], dt.float32, tag="w")
                    nc.scalar.activation(rn[:], hgx[:], AF.Relu, scale=-1.0)
                    em = work.tile([FP, VL], dt.float32, tag="w")
                    nc.scalar.activation(em[:], rn[:], AF.Exp, scale=-1.0)
                    nc.vector.scalar_tensor_tensor(
                        out=HE[g][:], in0=r0[:], scalar=-1.0, in1=em[:],
                        op0=op.add, op1=op.add)
                    # this graph's fuse leg (PE+Act only)
                    ai_ps = ps_m.tile([1, VL], dt.float32, tag="m")
                    nc.tensor.matmul(ai_ps[:], QGs[l][:, g:g + 1], HE[g][:],
                                     start=True, stop=True)
                    e = smp.tile([1, VL], bf, tag="s")
                    nc.scalar.activation(e[:], ai_ps[:], AF.Exp)
                    return e

                # software-pipelined: g0 core first, its epilogue DVE work
                # threaded between g1 core groups; last groups small so the
                # final accumulation (and epilogue) finishes early
                CGS = [(0, 8, 0), (8, 8, 0), (16, 8, 0)]
                stwh_group(0)
                P_aggs = [ps_agg.tile([FP, VL], dt.float32, tag="agg",
                                      name=f"pagg{g}")
                          for g in range(2)]
                for j, (k0, cg, npool) in enumerate(CGS):
                    if j + 1 < NG:
                        stwh_group(j + 1)
                    core_group(0, k0, cg, P_aggs[0], npool)
                ncs0, den0 = epi_header(0, P_aggs[0])
                core_group(1, *CGS[0][:2], P_aggs[1], CGS[0][2])
                core_group(1, *CGS[1][:2], P_aggs[1], CGS[1][2])
                e0 = epi_tail(0, ncs0, den0)
                core_group(1, *CGS[2][:2], P_aggs[1], CGS[2][2])
                ncs1, den1 = epi_header(1, P_aggs[1])
                e1 = epi_tail(1, ncs1, den1)

                # gated fuse
                ei = [e0, e1]
                dsum = smp.tile([1, VL], dt.float32, tag="s")
                nc.vector.tensor_tensor(out=dsum[:], in0=ei[0][:],
                                        in1=ei[1][:], op=op.add)
                rds = smp.tile([1, VL], dt.float32, tag="s")
                nc.vector.reciprocal(rds[:], dsum[:])
                b0 = smp.tile([1, VL], bf, tag="s")
                nc.vector.tensor_tensor(out=b0[:], in0=ei[0][:], in1=rds[:],
                                        op=op.mult)
                bib_ps = ps_m.tile([FP, VL], dt.float32, tag="m")
                nc.tensor.matmul(bib_ps[:], ONE1s[:], b0[:],
                                 start=True, stop=True)
                dd = work.tile([FP, VL], bf, tag="w")
                nc.vector.tensor_tensor(out=dd[:], in0=HE[0][:], in1=HE[1][:],
                                        op=op.subtract)
                bd = work.tile([FP, VL], bf, tag="w")
                nc.vector.tensor_tensor(out=bd[:], in0=dd[:], in1=bib_ps[:],
                                        op=op.mult)
                if hf_out is None:
                    return bd          # caller folds HE[1] + bd linearly
                nc.vector.tensor_tensor(out=hf_out[:], in0=HE[1][:],
                                        in1=bd[:], op=op.add)

            # ---------------- hop 1 ----------------
            gat_prologueA(0, XOWN)
            gat_main(0, xT, HF1)

            # all-gather H1 (feature-major, bf16); hop-2 own-slice prologue
            # runs under the collective (depends only on HF1).
            ag_in = drp.tile([FP, VL], bf)
            ag_out = drp.tile([NCORES, FP, VL], bf)
            nc.sync.dma_start(out=ag_in[:], in_=HF1[:])
            agv = ag_out.opt().rearrange("c (f v) -> c f v", v=VL)
            h1v = H1T.rearrange("f (c v) -> f c v", v=VL)
            if NO_COLLECTIVE:
                for c in range(NCORES):
                    nc.sync.dma_start(out=agv[c], in_=ag_in[:])
                    nc.sync.dma_start(out=h1v[:, c, :], in_=agv[c])
            else:
                nc.gpsimd.collective_compute(
                    "AllGather", op.bypass,
                    replica_groups=[list(range(NCORES))],
                    ins=[ag_in.opt()], outs=[ag_out.opt()])
                for c in range(NCORES):
                    nc.sync.dma_start(out=h1v[:, c, :], in_=agv[c])
            gat_prologueA(1, HF1)

            # ---------------- hop 2 + MLP head ----------------
            # H2 = HE[1] + bd is folded into the first MLP matmul (linear)
            bd2 = gat_main(1, H1T, None)
            h_ps = ps_m.tile([MH, VL], dt.float32, tag="m")
            nc.tensor.matmul(h_ps[:], MW1[:], HE[1][:], start=True, stop=False)
            nc.tensor.matmul(h_ps[:], MW1[:], bd2[:], start=False, stop=True)
            hd = smp.tile([MH, VL], bf, tag="s")
            nc.vector.tensor_scalar(hd[:], h_ps[:], MB1[:], 0.0,
                                    op.add, op.max)
            o_ps = ps_m.tile([1, VL], dt.float32, tag="m")
            nc.tensor.matmul(o_ps[:], MW2[:], hd[:], start=True, stop=True)
            osb = smp.tile([1, VL], dt.float32, tag="s")
            nc.vector.tensor_scalar(osb[:], o_ps[:], MB2[:], None,
                                    op.add, op.bypass)
            nc.sync.dma_start(out=out_d.ap(), in_=osb[:])

    nc.compile()
    return nc


def _pad_rows(w):
    out = np.zeros((FP,) + w.shape[1:], dtype=np.float32)
    for h in range(HEADS):
        out[BLK * h:BLK * h + HD] = w[HD * h:HD * h + HD]
    return out


def _ahat(a):
    A = np.zeros((HID, 2 * HEADS), dtype=np.float32)
    for h in range(HEADS):
        A[HD * h:HD * h + HD, h] = a[h, :HD]
        A[HD * h:HD * h + HD, HEADS + h] = a[h, HD:]
    return A


def _prep_adj(adj, c):
    """(N,N) int -> per-core (P, UC*VL) bf16 {0,1} chunk layout of adjT."""
    sl = adj[c * VL:(c + 1) * VL, :].T.astype(np.float32)       # (N, VL)
    sl = sl.reshape(UC, P, VL).transpose(1, 0, 2).reshape(P, UC * VL)
    return np.ascontiguousarray(sl).astype(ml_dtypes.bfloat16)


def _bf(x):
    return np.ascontiguousarray(x).astype(ml_dtypes.bfloat16)


def kernel(**inputs):
    from concourse.bass_utils import run_bass_kernel_spmd

    if "nc" not in _CACHE:
        _CACHE["nc"] = _build()
    nc = _CACHE["nc"]

    f32 = np.float32
    x = np.asarray(inputs["x"], f32)
    adj = [np.asarray(inputs["adj_ind"]), np.asarray(inputs["adj_cor"])]
    W1 = [np.asarray(inputs["W1i"], f32), np.asarray(inputs["W1c"], f32)]
    W2 = [np.asarray(inputs["W2i"], f32), np.asarray(inputs["W2c"], f32)]
    A1 = [np.asarray(inputs["a1i"], f32), np.asarray(inputs["a1c"], f32)]
    A2 = [np.asarray(inputs["a2i"], f32), np.asarray(inputs["a2c"], f32)]
    q1 = [np.asarray(inputs["q1i"], f32), np.asarray(inputs["q1c"], f32)]
    q2 = [np.asarray(inputs["q2i"], f32), np.asarray(inputs["q2c"], f32)]

    common = {"xT": _bf(x.T)}
    for l, (Ws, As) in enumerate(((W1, A1), (W2, A2))):
        blocks = []
        for g in range(2):
            W = Ws[g] if l == 0 else _pad_rows(Ws[g])
            WA = W @ _ahat(As[g])
            blocks.append(np.hstack([W, WA]))
        common[f"WST{l}"] = _bf(np.hstack(blocks))
    for l, qs in enumerate((q1, q2)):
        common[f"qg{l}"] = _bf(
            np.stack([_pad_rows(qs[0][:, None])[:, 0],
                      _pad_rows(qs[1][:, None])[:, 0]], axis=1))
    common["mw1"] = _bf(_pad_rows(np.asarray(inputs["mlp_w1"], f32)))
    common["mb1"] = np.ascontiguousarray(
        np.asarray(inputs["mlp_b1"], f32)[:, None])
    common["mw2"] = _bf(np.asarray(inputs["mlp_w2"], f32))
    common["mb2"] = np.asarray(inputs["mlp_b2"], f32).reshape(1, 1)

    in_maps = []
    for c in range(NCORES):
        m = dict(common)
        m["xOwnT"] = _bf(x[c * VL:(c + 1) * VL, :].T)
        m["adjTB_i"] = _prep_adj(adj[0], c)
        m["adjTB_c"] = _prep_adj(adj[1], c)
        in_maps.append(m)

    res = run_bass_kernel_spmd(nc, in_maps, core_ids=list(range(NCORES)))
    out = np.concatenate([r["out"][0] for r in res.results])[:, None]
    return out.astype(np.float32)


if __name__ == "__main__":
    _CACHE["nc"] = _build()
    print("build ok")


# revision 31
# speedup vs baseline: 1.1010x; 1.0307x over previous
"""DualGAT (2-hop, 2-graph GAT + gated fuse + MLP) on 8 Trainium2 NeuronCores.

Math per GAT layer/head (z[v,u] = s_v + t_u):
    exp(LeakyRelu(z, 0.2)) = p_v q_u max(exp(0.8 z), 1)      (exact)
    exp(0.8 z) = R_v r_u                                      (separable)
with p = exp(0.2 s), q = exp(0.2 t), R = exp(0.8 s), r = exp(0.8 t).
The p_v factor is common to numerator and denominator and cancels, so
    H[v] = (sum_u adj * w * q_u [Wh_u|1]) / den,  w = max(R_v r_u, 1)
One 4x-mode DVE tensor_scalar per (chunk, head) builds w = (R_b * r) max 1,
one 2x TT per 8-chunk group applies the adjacency mask in place, and a
single M=32 bf16 matmul per (chunk, head) accumulates num|den into PSUM.

Sharding: v (attention rows) split 8 ways, 384 rows/core; u (neighbors) full.
Feature tensors use padded 4x32 head blocks (col 16 = softmax denominator,
~1.0 junk after normalize); downstream weight rows are zero-padded there.
"""

import sys
import numpy as np

for _p in ("/opt/trn_rl_repo",):
    if _p not in sys.path:
        sys.path.insert(0, _p)

import ml_dtypes

N = 3072
IN_DIM = 32
HID = 64
HEADS = 4
HD = 16
NCORES = 8
VL = N // NCORES          # 384
P = 128
UC = N // P               # 24
FP = 128                  # padded feature rows: 4 heads x 32
MH = HID // 2
KROWS = [IN_DIM, FP]
BLK = 32
GOFF = [0, 72]
SOFF = [64, 136]
TOFF = [68, 140]
CG = 8                    # chunks per mask group
SWG = 8                   # chunks per stwh/GW-build group

NO_COLLECTIVE = False

_CACHE = {}


def _build():
    import concourse.bacc as bacc
    import concourse.mybir as mybir
    from concourse.tile import TileContext

    dt = mybir.dt
    op = mybir.AluOpType
    AF = mybir.ActivationFunctionType
    bf = dt.bfloat16

    nc = bacc.Bacc("TRN2", target_bir_lowering=False, debug=False,
                   num_devices=NCORES)

    def dram_in(name, shape, dtype=bf):
        return nc.dram_tensor(name, list(shape), dtype, kind="ExternalInput")

    xT_d = dram_in("xT", (IN_DIM, N))
    xOwn_d = dram_in("xOwnT", (IN_DIM, VL))
    adj_d = [dram_in("adjTB_i", (P, UC * VL)),
             dram_in("adjTB_c", (P, UC * VL))]
    WST_d = [dram_in(f"WST{l}", (KROWS[l], 144)) for l in range(2)]
    qg_d = [dram_in(f"qg{l}", (FP, 2)) for l in range(2)]
    mw1_d = dram_in("mw1", (FP, MH))
    mb1_d = dram_in("mb1", (MH, 1), dt.float32)
    mw2_d = dram_in("mw2", (MH, 1))
    mb2_d = dram_in("mb2", (1, 1), dt.float32)
    out_d = nc.dram_tensor("out", [1, VL], dt.float32, kind="ExternalOutput")

    # inline consts
    sel4_np = np.zeros((HEADS, HEADS * P), dtype=np.float32)
    for h in range(HEADS):
        sel4_np[h, P * h:P * (h + 1)] = 1.0
    sel4_d = nc.inline_tensor(sel4_np.astype(ml_dtypes.bfloat16), name="sel4")
    e17_np = np.zeros((HEADS, FP), dtype=np.float32)
    for h in range(HEADS):
        e17_np[h, BLK * h:BLK * (h + 1)] = 1.0
    e17_d = nc.inline_tensor(e17_np.astype(ml_dtypes.bfloat16), name="e17")
    one1_d = nc.inline_tensor(np.ones((1, P), dtype=np.float32)
                              .astype(ml_dtypes.bfloat16), name="one1")

    def sb(name, shape, dtype=dt.float32):
        return nc.alloc_sbuf_tensor(name, list(shape), dtype).ap()

    xT = sb("s_xT", (IN_DIM, N), bf)
    XOWN = sb("s_xOwn", (IN_DIM, VL), bf)
    adjTB = [sb(f"s_adjTB{g}", (P, UC * VL), bf) for g in range(2)]
    WSTs = [sb(f"s_WST{l}", (KROWS[l], 144), bf) for l in range(2)]
    WH = sb("s_WH", (P, UC * 144), bf)
    H1T = sb("s_H1T", (FP, N), bf)
    GW = [sb(f"s_GW{g}", (P, UC * HEADS * BLK), bf) for g in range(2)]
    RB4 = [sb(f"s_RB4_{g}", (P, HEADS * VL), bf) for g in range(2)]
    RCOL = [sb(f"s_RCOL{g}", (P, UC * HEADS)) for g in range(2)]
    QCOL = [sb(f"s_QCOL{g}", (P, UC * HEADS)) for g in range(2)]
    RRB = [sb(f"s_RRB{g}", (HEADS, VL), bf) for g in range(2)]
    HE = [sb(f"s_HE{g}", (FP, VL), bf) for g in range(2)]
    HF1 = sb("s_HF1", (FP, VL), bf)
    HF2 = sb("s_HF2", (FP, VL), bf)
    SEL4s = sb("s_sel4", (HEADS, HEADS * P), bf)
    E17s = sb("s_e17", (HEADS, FP), bf)
    ONE1s = sb("s_one1", (1, P), bf)
    QGs = [sb(f"s_qg{l}", (FP, 2), bf) for l in range(2)]
    MW1 = sb("s_mw1", (FP, MH), bf)
    MB1 = sb("s_mb1", (MH, 1))
    MW2 = sb("s_mw2", (MH, 1), bf)
    MB2 = sb("s_mb2", (1, 1))

    WH_v = WH.rearrange("p (k c) -> p k c", c=144)
    adj_v = [a.rearrange("p (k v) -> p k v", v=VL) for a in adjTB]
    GW_v = [g.rearrange("p (k h c) -> p k h c", h=HEADS, c=BLK) for g in GW]
    RB4_v = [r.rearrange("p (h v) -> p h v", h=HEADS) for r in RB4]
    QCOL_v = [q.rearrange("p (k h) -> p k h", h=HEADS) for q in QCOL]
    RCOL_v = [r.rearrange("p (k h) -> p k h", h=HEADS) for r in RCOL]

    with TileContext(nc) as tc:
        with tc.tile_pool(name="w8p", bufs=3) as wp, \
             tc.tile_pool(name="work", bufs=5) as work, \
             tc.tile_pool(name="small", bufs=6) as smp, \
             tc.tile_pool(name="ps_w", bufs=2, space="PSUM") as ps_w, \
             tc.tile_pool(name="ps_agg", bufs=2, space="PSUM") as ps_agg, \
             tc.tile_pool(name="ps_m", bufs=3, space="PSUM") as ps_m, \
             tc.tile_pool(name="dram", bufs=1, space="DRAM") as drp:

            # ---------- loads: adjacency first (largest, needed by hop-1) ----
            nc.sync.dma_start(out=xT[:], in_=xT_d.ap())
            nc.sync.dma_start(out=XOWN[:], in_=xOwn_d.ap())
            for l in range(2):
                nc.sync.dma_start(out=WSTs[l][:], in_=WST_d[l].ap())
            nc.sync.dma_start(out=SEL4s[:], in_=sel4_d.ap())
            adjg_d = [a.ap().rearrange("p (k v) -> p k v", v=VL) for a in adj_d]
            for g in range(2):
                nc.sync.dma_start(out=adj_v[g][:, 0:SWG, :],
                                  in_=adjg_d[g][:, 0:SWG, :])
            nc.sync.dma_start(out=E17s[:], in_=e17_d.ap())
            nc.sync.dma_start(out=ONE1s[:], in_=one1_d.ap())
            for l in range(2):
                nc.sync.dma_start(out=QGs[l][:], in_=qg_d[l].ap())
            for i in range(1, UC // SWG):
                sl = slice(i * SWG, (i + 1) * SWG)
                for g in range(2):
                    nc.sync.dma_start(out=adj_v[g][:, sl, :],
                                      in_=adjg_d[g][:, sl, :])
            nc.sync.dma_start(out=MW1[:], in_=mw1_d.ap())
            nc.sync.dma_start(out=MB1[:], in_=mb1_d.ap())
            nc.sync.dma_start(out=MW2[:], in_=mw2_d.ap())
            nc.sync.dma_start(out=MB2[:], in_=mb2_d.ap())
            for g in range(2):
                nc.gpsimd.memset(GW_v[g][:, :, :, HD + 1:BLK], 0.0)

            def gat_prologueA(l, hown):
                """Own-slice path: R = exp(0.8 s) broadcast into RB4 (bf16).
                Depends only on hown + weights, not on the all-gathered HT."""
                krows = KROWS[l]
                wst = WSTs[l]
                for g in range(2):
                    st_ps = ps_m.tile([8, VL], dt.float32, tag="m")
                    nc.tensor.matmul(st_ps[:], wst[0:krows, SOFF[g]:SOFF[g] + 8],
                                     hown[:], start=True, stop=True)
                    nc.scalar.activation(RRB[g][:], st_ps[0:HEADS, :], AF.Exp,
                                         scale=0.8)
                    for h in range(HEADS):
                        rps = ps_m.tile([P, VL], dt.float32, tag="m")
                        nc.tensor.matmul(rps[:], SEL4s[:, P * h:P * (h + 1)],
                                         RRB[g][:], start=True, stop=True)
                        nc.vector.tensor_copy(out=RB4_v[g][:, h, :],
                                              in_=rps[:])

            def gat_main(l, HT, hf_out):
                krows = KROWS[l]
                wst = WSTs[l]
                NG = UC // SWG

                def stwh_group(i):
                    sl = slice(i * SWG, (i + 1) * SWG)
                    for k in range(i * SWG, (i + 1) * SWG):
                        stwh = ps_w.tile([P, 144], dt.float32, tag="stwh")
                        nc.tensor.matmul(stwh[:], HT[:, P * k:P * (k + 1)],
                                         wst[0:krows, :], start=True, stop=True)
                        nc.scalar.copy(WH_v[:, k, :], stwh[:])
                    for g in range(2):
                        tcols = WH_v[:, sl, TOFF[g]:TOFF[g] + HEADS]
                        nc.scalar.activation(QCOL_v[g][:, sl, :], tcols,
                                             AF.Exp, scale=0.2)
                        nc.scalar.activation(RCOL_v[g][:, sl, :], tcols,
                                             AF.Exp, scale=0.8)
                        nc.gpsimd.tensor_tensor(
                            out=GW_v[g][:, sl, :, 0:HD],
                            in0=WH_v[:, sl, GOFF[g]:GOFF[g] + HID].rearrange(
                                "p k (h d) -> p k h d", d=HD),
                            in1=QCOL_v[g][:, sl, :, None].to_broadcast(
                                (P, SWG, HEADS, HD)),
                            op=op.mult)
                        nc.gpsimd.tensor_copy(out=GW_v[g][:, sl, :, HD],
                                              in_=QCOL_v[g][:, sl, :])

                def core_group(g, k0, cg, P_agg, npool):
                    """cg chunks from k0; first npool chunks' weight TSPs
                    run on Pool to offload DVE."""
                    w8f = wp.tile([P, CG, HEADS, VL], bf, tag="w8")
                    w8 = w8f[:, 0:cg]
                    for c in range(cg):
                        k = k0 + c
                        eng = nc.gpsimd if c < npool else nc.vector
                        for h in range(HEADS):
                            eng.tensor_scalar(
                                w8[:, c, h, :], RB4_v[g][:, h, :],
                                RCOL[g][:, k * HEADS + h:k * HEADS + h + 1],
                                1.0, op.mult, op.max)
                    nc.vector.tensor_tensor(
                        out=w8[:], in0=w8[:],
                        in1=adj_v[g][:, k0:k0 + cg, :][:, :, None, :]
                            .to_broadcast((P, cg, HEADS, VL)),
                        op=op.mult)
                    for c in range(cg):
                        k = k0 + c
                        for h in range(HEADS):
                            nc.tensor.matmul(
                                P_agg[BLK * h:BLK * h + BLK, :],
                                GW_v[g][:, k, h, :], w8[:, c, h, :],
                                start=(k == 0), stop=(k == UC - 1),
                                tile_position=(0, BLK * h))

                def epi_header(g, P_agg):
                    """Act copy + den gather DMA — no DVE ops."""
                    ncs = work.tile([FP, VL], bf, tag="w", name=f"ncs{g}")
                    nc.scalar.copy(ncs[:], P_agg[:])
                    den4 = smp.tile([HEADS, VL], bf, tag="s", name=f"den{g}")
                    nc.sync.dma_start(out=den4[:], in_=ncs[HD::BLK, :])
                    return ncs, den4

                def epi_tail(g, ncs, den4):
                    rdf = smp.tile([HEADS, VL], bf, tag="s")
                    with nc.allow_low_precision(reason="den recip, 0.4% ok"):
                        nc.vector.reciprocal(rdf[:], den4[:])
                    rb_ps = ps_m.tile([FP, VL], dt.float32, tag="m")
                    nc.tensor.matmul(rb_ps[:], E17s[:], rdf[:],
                                     start=True, stop=True)
                    hgx = work.tile([FP, VL], dt.float32, tag="w")
                    nc.vector.tensor_tensor(out=hgx[:], in0=ncs[:],
                                            in1=rb_ps[:], op=op.mult)
                    r0 = work.tile([FP, VL], dt.float32, tag="w")
                    nc.scalar.activation(r0[:], hgx[:], AF.Relu)
                    rn = work.tile([FP, VL], dt.float32, tag="w")
                    nc.scalar.activation(rn[:], hgx[:], AF.Relu, scale=-1.0)
                    em = work.tile([FP, VL], dt.float32, tag="w")
                    nc.scalar.activation(em[:], rn[:], AF.Exp, scale=-1.0)
                    nc.vector.scalar_tensor_tensor(
                        out=HE[g][:], in0=r0[:], scalar=-1.0, in1=em[:],
                        op0=op.add, op1=op.add)
                    # this graph's fuse leg (PE+Act only)
                    ai_ps = ps_m.tile([1, VL], dt.float32, tag="m")
                    nc.tensor.matmul(ai_ps[:], QGs[l][:, g:g + 1], HE[g][:],
                                     start=True, stop=True)
                    e = smp.tile([1, VL], bf, tag="s")
                    nc.scalar.activation(e[:], ai_ps[:], AF.Exp)
                    return e

                # software-pipelined: g0 core first, its epilogue DVE work
                # threaded between g1 core groups
                CGS = [(i * CG, CG) for i in range(UC // CG)]
                stwh_group(0)
                P_aggs = [ps_agg.tile([FP, VL], dt.float32, tag="agg",
                                      name=f"pagg{g}")
                          for g in range(2)]
                for k0, cg in CGS:
                    if k0 % SWG == 0 and k0 // SWG + 1 < NG:
                        stwh_group(k0 // SWG + 1)
                    core_group(0, k0, cg, P_aggs[0], 0)
                ncs0, den0 = epi_header(0, P_aggs[0])
                esplit = max(1, (2 * len(CGS)) // 3)
                for k0, cg in CGS[:esplit]:
                    core_group(1, k0, cg, P_aggs[1], 0)
                e0 = epi_tail(0, ncs0, den0)
                for k0, cg in CGS[esplit:]:
                    core_group(1, k0, cg, P_aggs[1], 0)
                ncs1, den1 = epi_header(1, P_aggs[1])
                e1 = epi_tail(1, ncs1, den1)

                # gated fuse
                ei = [e0, e1]
                dsum = smp.tile([1, VL], dt.float32, tag="s")
                nc.vector.tensor_tensor(out=dsum[:], in0=ei[0][:],
                                        in1=ei[1][:], op=op.add)
                rds = smp.tile([1, VL], dt.float32, tag="s")
                nc.vector.reciprocal(rds[:], dsum[:])
                b0 = smp.tile([1, VL], bf, tag="s")
                nc.vector.tensor_tensor(out=b0[:], in0=ei[0][:], in1=rds[:],
                                        op=op.mult)
                bib_ps = ps_m.tile([FP, VL], dt.float32, tag="m")
                nc.tensor.matmul(bib_ps[:], ONE1s[:], b0[:],
                                 start=True, stop=True)
                dd = work.tile([FP, VL], bf, tag="w")
                nc.vector.tensor_tensor(out=dd[:], in0=HE[0][:], in1=HE[1][:],
                                        op=op.subtract)
                bd = work.tile([FP, VL], bf, tag="w")
                nc.vector.tensor_tensor(out=bd[:], in0=dd[:], in1=bib_ps[:],
                                        op=op.mult)
                if hf_out is None:
                    return bd          # caller folds HE[1] + bd linearly
                nc.vector.tensor_tensor(out=hf_out[:], in0=HE[1][:],
                                        in1=bd[:], op=op.add)

            # ---------------- hop 1 ----------------
            gat_prologueA(0, XOWN)
            gat_main(0, xT, HF1)

            # all-gather H1 (feature-major, bf16); hop-2 own-slice prologue
            # runs under the collective (depends only on HF1).
            ag_in = drp.tile([FP, VL], bf)
            ag_out = drp.tile([NCORES, FP, VL], bf)
            nc.sync.dma_start(out=ag_in[:], in_=HF1[:])
            agv = ag_out.opt().rearrange("c (f v) -> c f v", v=VL)
            h1v = H1T.rearrange("f (c v) -> f c v", v=VL)
            if NO_COLLECTIVE:
                for c in range(NCORES):
                    nc.sync.dma_start(out=agv[c], in_=ag_in[:])
                    nc.sync.dma_start(out=h1v[:, c, :], in_=agv[c])
            else:
                nc.gpsimd.collective_compute(
                    "AllGather", op.bypass,
                    replica_groups=[list(range(NCORES))],
                    ins=[ag_in.opt()], outs=[ag_out.opt()])
                for c in range(NCORES):
                    nc.sync.dma_start(out=h1v[:, c, :], in_=agv[c])
            gat_prologueA(1, HF1)

            # ---------------- hop 2 + MLP head ----------------
            # H2 = HE[1] + bd is folded into the first MLP matmul (linear)
            bd2 = gat_main(1, H1T, None)
            h_ps = ps_m.tile([MH, VL], dt.float32, tag="m")
            nc.tensor.matmul(h_ps[:], MW1[:], HE[1][:], start=True, stop=False)
            nc.tensor.matmul(h_ps[:], MW1[:], bd2[:], start=False, stop=True)
            hd = smp.tile([MH, VL], bf, tag="s")
            nc.vector.tensor_scalar(hd[:], h_ps[:], MB1[:], 0.0,
                                    op.add, op.max)
            o_ps = ps_m.tile([1, VL], dt.float32, tag="m")
            nc.tensor.matmul(o_ps[:], MW2[:], hd[:], start=True, stop=True)
            osb = smp.tile([1, VL], dt.float32, tag="s")
            nc.vector.tensor_scalar(osb[:], o_ps[:], MB2[:], None,
                                    op.add, op.bypass)
            nc.sync.dma_start(out=out_d.ap(), in_=osb[:])

    nc.compile()
    return nc


def _pad_rows(w):
    out = np.zeros((FP,) + w.shape[1:], dtype=np.float32)
    for h in range(HEADS):
        out[BLK * h:BLK * h + HD] = w[HD * h:HD * h + HD]
    return out


def _ahat(a):
    A = np.zeros((HID, 2 * HEADS), dtype=np.float32)
    for h in range(HEADS):
        A[HD * h:HD * h + HD, h] = a[h, :HD]
        A[HD * h:HD * h + HD, HEADS + h] = a[h, HD:]
    return A


def _prep_adj(adj, c):
    """(N,N) int -> per-core (P, UC*VL) bf16 {0,1} chunk layout of adjT."""
    sl = adj[c * VL:(c + 1) * VL, :].T.astype(np.float32)       # (N, VL)
    sl = sl.reshape(UC, P, VL).transpose(1, 0, 2).reshape(P, UC * VL)
    return np.ascontiguousarray(sl).astype(ml_dtypes.bfloat16)


def _bf(x):
    return np.ascontiguousarray(x).astype(ml_dtypes.bfloat16)


def kernel(**inputs):
    from concourse.bass_utils import run_bass_kernel_spmd

    if "nc" not in _CACHE:
        _CACHE["nc"] = _build()
    nc = _CACHE["nc"]

    f32 = np.float32
    x = np.asarray(inputs["x"], f32)
    adj = [np.asarray(inputs["adj_ind"]), np.asarray(inputs["adj_cor"])]
    W1 = [np.asarray(inputs["W1i"], f32), np.asarray(inputs["W1c"], f32)]
    W2 = [np.asarray(inputs["W2i"], f32), np.asarray(inputs["W2c"], f32)]
    A1 = [np.asarray(inputs["a1i"], f32), np.asarray(inputs["a1c"], f32)]
    A2 = [np.asarray(inputs["a2i"], f32), np.asarray(inputs["a2c"], f32)]
    q1 = [np.asarray(inputs["q1i"], f32), np.asarray(inputs["q1c"], f32)]
    q2 = [np.asarray(inputs["q2i"], f32), np.asarray(inputs["q2c"], f32)]

    common = {"xT": _bf(x.T)}
    for l, (Ws, As) in enumerate(((W1, A1), (W2, A2))):
        blocks = []
        for g in range(2):
            W = Ws[g] if l == 0 else _pad_rows(Ws[g])
            WA = W @ _ahat(As[g])
            blocks.append(np.hstack([W, WA]))
        common[f"WST{l}"] = _bf(np.hstack(blocks))
    for l, qs in enumerate((q1, q2)):
        common[f"qg{l}"] = _bf(
            np.stack([_pad_rows(qs[0][:, None])[:, 0],
                      _pad_rows(qs[1][:, None])[:, 0]], axis=1))
    common["mw1"] = _bf(_pad_rows(np.asarray(inputs["mlp_w1"], f32)))
    common["mb1"] = np.ascontiguousarray(
        np.asarray(inputs["mlp_b1"], f32)[:, None])
    common["mw2"] = _bf(np.asarray(inputs["mlp_w2"], f32))
    common["mb2"] = np.asarray(inputs["mlp_b2"], f32).reshape(1, 1)

    in_maps = []
    for c in range(NCORES):
        m = dict(common)
        m["xOwnT"] = _bf(x[c * VL:(c + 1) * VL, :].T)
        m["adjTB_i"] = _prep_adj(adj[0], c)
        m["adjTB_c"] = _prep_adj(adj[1], c)
        in_maps.append(m)

    res = run_bass_kernel_spmd(nc, in_maps, core_ids=list(range(NCORES)))
    out = np.concatenate([r["out"][0] for r in res.results])[:, None]
    return out.astype(np.float32)


if __name__ == "__main__":
    _CACHE["nc"] = _build()
    print("build ok")


# revision 33
# speedup vs baseline: 1.1432x; 1.0383x over previous
"""DualGAT (2-hop, 2-graph GAT + gated fuse + MLP) on 8 Trainium2 NeuronCores.

Math per GAT layer/head (z[v,u] = s_v + t_u):
    exp(LeakyRelu(z, 0.2)) = p_v q_u max(exp(0.8 z), 1)      (exact)
    exp(0.8 z) = R_v r_u                                      (separable)
with p = exp(0.2 s), q = exp(0.2 t), R = exp(0.8 s), r = exp(0.8 t).
The p_v factor is common to numerator and denominator and cancels, so
    H[v] = (sum_u adj * w * q_u [Wh_u|1]) / den,  w = max(R_v r_u, 1)
One 4x-mode DVE tensor_scalar per (chunk, head) builds w = (R_b * r) max 1,
one 2x TT per 8-chunk group applies the adjacency mask in place, and a
single M=32 bf16 matmul per (chunk, head) accumulates num|den into PSUM.

Sharding: v (attention rows) split 8 ways, 384 rows/core; u (neighbors) full.
Feature tensors use padded 4x32 head blocks (col 16 = softmax denominator,
~1.0 junk after normalize); downstream weight rows are zero-padded there.
"""

import sys
import numpy as np

for _p in ("/opt/trn_rl_repo",):
    if _p not in sys.path:
        sys.path.insert(0, _p)

import ml_dtypes

N = 3072
IN_DIM = 32
HID = 64
HEADS = 4
HD = 16
NCORES = 8
VL = N // NCORES          # 384
P = 128
UC = N // P               # 24
FP = 128                  # padded feature rows: 4 heads x 32
MH = HID // 2
KROWS = [IN_DIM, FP]
BLK = 32
GOFF = [0, 72]
SOFF = [64, 136]
TOFF = [68, 140]
CG = 2                    # chunks per mask group
SWG = 8                   # chunks per stwh/GW-build group

NO_COLLECTIVE = False

_CACHE = {}


def _build():
    import concourse.bacc as bacc
    import concourse.mybir as mybir
    from concourse.tile import TileContext

    dt = mybir.dt
    op = mybir.AluOpType
    AF = mybir.ActivationFunctionType
    bf = dt.bfloat16

    nc = bacc.Bacc("TRN2", target_bir_lowering=False, debug=False,
                   num_devices=NCORES)

    def dram_in(name, shape, dtype=bf):
        return nc.dram_tensor(name, list(shape), dtype, kind="ExternalInput")

    xT_d = dram_in("xT", (IN_DIM, N))
    xOwn_d = dram_in("xOwnT", (IN_DIM, VL))
    adj_d = [dram_in("adjTB_i", (P, UC * VL)),
             dram_in("adjTB_c", (P, UC * VL))]
    WST_d = [dram_in(f"WST{l}", (KROWS[l], 144)) for l in range(2)]
    qg_d = [dram_in(f"qg{l}", (FP, 2)) for l in range(2)]
    mw1_d = dram_in("mw1", (FP, MH))
    mb1_d = dram_in("mb1", (MH, 1), dt.float32)
    mw2_d = dram_in("mw2", (MH, 1))
    mb2_d = dram_in("mb2", (1, 1), dt.float32)
    out_d = nc.dram_tensor("out", [1, VL], dt.float32, kind="ExternalOutput")

    # inline consts
    sel4_np = np.zeros((HEADS, HEADS * P), dtype=np.float32)
    for h in range(HEADS):
        sel4_np[h, P * h:P * (h + 1)] = 1.0
    sel4_d = nc.inline_tensor(sel4_np.astype(ml_dtypes.bfloat16), name="sel4")
    selden_np = np.zeros((FP, FP), dtype=np.float32)
    for p in range(FP):
        selden_np[HD + BLK * (p // BLK), p] = 1.0
    selden_d = nc.inline_tensor(selden_np.astype(ml_dtypes.bfloat16),
                                name="selden")
    one1_d = nc.inline_tensor(np.ones((1, P), dtype=np.float32)
                              .astype(ml_dtypes.bfloat16), name="one1")

    def sb(name, shape, dtype=dt.float32):
        return nc.alloc_sbuf_tensor(name, list(shape), dtype).ap()

    xT = sb("s_xT", (IN_DIM, N), bf)
    XOWN = sb("s_xOwn", (IN_DIM, VL), bf)
    adjTB = [sb(f"s_adjTB{g}", (P, UC * VL), bf) for g in range(2)]
    WSTs = [sb(f"s_WST{l}", (KROWS[l], 144), bf) for l in range(2)]
    WH = sb("s_WH", (P, UC * 144), bf)
    H1T = sb("s_H1T", (FP, N), bf)
    GW = [sb(f"s_GW{g}", (P, UC * HEADS * BLK), bf) for g in range(2)]
    RB4 = [sb(f"s_RB4_{g}", (P, HEADS * VL), bf) for g in range(2)]
    RCOL = [sb(f"s_RCOL{g}", (P, UC * HEADS)) for g in range(2)]
    QCOL = [sb(f"s_QCOL{g}", (P, UC * HEADS)) for g in range(2)]
    RRB = [sb(f"s_RRB{g}", (HEADS, VL), bf) for g in range(2)]
    HE = [sb(f"s_HE{g}", (FP, VL), bf) for g in range(2)]
    HF1 = sb("s_HF1", (FP, VL), bf)
    HF2 = sb("s_HF2", (FP, VL), bf)
    SEL4s = sb("s_sel4", (HEADS, HEADS * P), bf)
    SELDENs = sb("s_selden", (FP, FP), bf)
    ONE1s = sb("s_one1", (1, P), bf)
    QGs = [sb(f"s_qg{l}", (FP, 2), bf) for l in range(2)]
    MW1 = sb("s_mw1", (FP, MH), bf)
    MB1 = sb("s_mb1", (MH, 1))
    MW2 = sb("s_mw2", (MH, 1), bf)
    MB2 = sb("s_mb2", (1, 1))

    WH_v = WH.rearrange("p (k c) -> p k c", c=144)
    adj_v = [a.rearrange("p (k v) -> p k v", v=VL) for a in adjTB]
    GW_v = [g.rearrange("p (k h c) -> p k h c", h=HEADS, c=BLK) for g in GW]
    RB4_v = [r.rearrange("p (h v) -> p h v", h=HEADS) for r in RB4]
    QCOL_v = [q.rearrange("p (k h) -> p k h", h=HEADS) for q in QCOL]
    RCOL_v = [r.rearrange("p (k h) -> p k h", h=HEADS) for r in RCOL]

    with TileContext(nc) as tc:
        with tc.tile_pool(name="w8p", bufs=3) as wp, \
             tc.tile_pool(name="work", bufs=5) as work, \
             tc.tile_pool(name="small", bufs=6) as smp, \
             tc.tile_pool(name="ps_w", bufs=2, space="PSUM") as ps_w, \
             tc.tile_pool(name="ps_agg", bufs=2, space="PSUM") as ps_agg, \
             tc.tile_pool(name="ps_m", bufs=3, space="PSUM") as ps_m, \
             tc.tile_pool(name="dram", bufs=1, space="DRAM") as drp:

            # ---------- loads: adjacency first (largest, needed by hop-1) ----
            nc.sync.dma_start(out=xT[:], in_=xT_d.ap())
            nc.sync.dma_start(out=XOWN[:], in_=xOwn_d.ap())
            for l in range(2):
                nc.sync.dma_start(out=WSTs[l][:], in_=WST_d[l].ap())
            nc.sync.dma_start(out=SEL4s[:], in_=sel4_d.ap())
            adjg_d = [a.ap().rearrange("p (k v) -> p k v", v=VL) for a in adj_d]
            for g in range(2):
                nc.sync.dma_start(out=adj_v[g][:, 0:SWG, :],
                                  in_=adjg_d[g][:, 0:SWG, :])
            nc.sync.dma_start(out=SELDENs[:], in_=selden_d.ap())
            nc.sync.dma_start(out=ONE1s[:], in_=one1_d.ap())
            for l in range(2):
                nc.sync.dma_start(out=QGs[l][:], in_=qg_d[l].ap())
            for i in range(1, UC // SWG):
                sl = slice(i * SWG, (i + 1) * SWG)
                for g in range(2):
                    nc.sync.dma_start(out=adj_v[g][:, sl, :],
                                      in_=adjg_d[g][:, sl, :])
            nc.sync.dma_start(out=MW1[:], in_=mw1_d.ap())
            nc.sync.dma_start(out=MB1[:], in_=mb1_d.ap())
            nc.sync.dma_start(out=MW2[:], in_=mw2_d.ap())
            nc.sync.dma_start(out=MB2[:], in_=mb2_d.ap())
            for g in range(2):
                nc.gpsimd.memset(GW_v[g][:, :, :, HD + 1:BLK], 0.0)

            def gat_prologueA(l, hown):
                """Own-slice path: R = exp(0.8 s) broadcast into RB4 (bf16).
                Depends only on hown + weights, not on the all-gathered HT."""
                krows = KROWS[l]
                wst = WSTs[l]
                for g in range(2):
                    st_ps = ps_m.tile([8, VL], dt.float32, tag="m")
                    nc.tensor.matmul(st_ps[:], wst[0:krows, SOFF[g]:SOFF[g] + 8],
                                     hown[:], start=True, stop=True)
                    nc.scalar.activation(RRB[g][:], st_ps[0:HEADS, :], AF.Exp,
                                         scale=0.8)
                    for h in range(HEADS):
                        rps = ps_m.tile([P, VL], dt.float32, tag="m")
                        nc.tensor.matmul(rps[:], SEL4s[:, P * h:P * (h + 1)],
                                         RRB[g][:], start=True, stop=True)
                        nc.scalar.copy(RB4_v[g][:, h, :], rps[:])

            def gat_main(l, HT, hf_out):
                krows = KROWS[l]
                wst = WSTs[l]
                NG = UC // SWG

                def stwh_group(i):
                    sl = slice(i * SWG, (i + 1) * SWG)
                    for k in range(i * SWG, (i + 1) * SWG):
                        stwh = ps_w.tile([P, 144], dt.float32, tag="stwh")
                        nc.tensor.matmul(stwh[:], HT[:, P * k:P * (k + 1)],
                                         wst[0:krows, :], start=True, stop=True)
                        if i == 0 and k % 2 == 0:
                            nc.vector.tensor_copy(out=WH_v[:, k, :],
                                                  in_=stwh[:])
                        else:
                            nc.scalar.copy(WH_v[:, k, :], stwh[:])
                    for g in range(2):
                        tcols = WH_v[:, sl, TOFF[g]:TOFF[g] + HEADS]
                        nc.scalar.activation(QCOL_v[g][:, sl, :], tcols,
                                             AF.Exp, scale=0.2)
                        nc.scalar.activation(RCOL_v[g][:, sl, :], tcols,
                                             AF.Exp, scale=0.8)
                        nc.gpsimd.tensor_tensor(
                            out=GW_v[g][:, sl, :, 0:HD],
                            in0=WH_v[:, sl, GOFF[g]:GOFF[g] + HID].rearrange(
                                "p k (h d) -> p k h d", d=HD),
                            in1=QCOL_v[g][:, sl, :, None].to_broadcast(
                                (P, SWG, HEADS, HD)),
                            op=op.mult)
                        nc.gpsimd.tensor_copy(out=GW_v[g][:, sl, :, HD],
                                              in_=QCOL_v[g][:, sl, :])

                def core_group(g, k0, cg, P_agg, npool):
                    """cg chunks from k0; first npool chunks' weight TSPs
                    run on Pool to offload DVE."""
                    w8f = wp.tile([P, CG, HEADS, VL], bf, tag="w8")
                    w8 = w8f[:, 0:cg]
                    for c in range(cg):
                        k = k0 + c
                        eng = nc.gpsimd if c < npool else nc.vector
                        for h in range(HEADS):
                            eng.tensor_scalar(
                                w8[:, c, h, :], RB4_v[g][:, h, :],
                                RCOL[g][:, k * HEADS + h:k * HEADS + h + 1],
                                1.0, op.mult, op.max)
                    nc.vector.tensor_tensor(
                        out=w8[:], in0=w8[:],
                        in1=adj_v[g][:, k0:k0 + cg, :][:, :, None, :]
                            .to_broadcast((P, cg, HEADS, VL)),
                        op=op.mult)
                    for c in range(cg):
                        k = k0 + c
                        for h in range(HEADS):
                            nc.tensor.matmul(
                                P_agg[BLK * h:BLK * h + BLK, :],
                                GW_v[g][:, k, h, :], w8[:, c, h, :],
                                start=(k == 0), stop=(k == UC - 1),
                                tile_position=(0, BLK * h))

                def epi_header(g, P_agg):
                    """Act copy only — no DVE ops."""
                    ncs = work.tile([FP, VL], bf, tag="w", name=f"ncs{g}")
                    nc.scalar.copy(ncs[:], P_agg[:])
                    return ncs, None

                def epi_tail(g, ncs, _unused):
                    # den broadcast via select-matmul, then reciprocal + mult
                    db_ps = ps_m.tile([FP, VL], dt.float32, tag="m")
                    nc.tensor.matmul(db_ps[:], SELDENs[:], ncs[:],
                                     start=True, stop=True)
                    rdb = work.tile([FP, VL], bf, tag="w")
                    with nc.allow_low_precision(reason="den recip, 0.4% ok"):
                        nc.vector.reciprocal(rdb[:], db_ps[:])
                    hgx = work.tile([FP, VL], bf, tag="w")
                    nc.vector.tensor_tensor(out=hgx[:], in0=ncs[:],
                                            in1=rdb[:], op=op.mult)
                    r0 = work.tile([FP, VL], dt.float32, tag="w")
                    nc.scalar.activation(r0[:], hgx[:], AF.Relu)
                    rn = work.tile([FP, VL], dt.float32, tag="w")
                    nc.scalar.activation(rn[:], hgx[:], AF.Relu, scale=-1.0)
                    em = work.tile([FP, VL], dt.float32, tag="w")
                    nc.scalar.activation(em[:], rn[:], AF.Exp, scale=-1.0)
                    nc.vector.scalar_tensor_tensor(
                        out=HE[g][:], in0=r0[:], scalar=-1.0, in1=em[:],
                        op0=op.add, op1=op.add)
                    # this graph's fuse leg (PE+Act only)
                    ai_ps = ps_m.tile([1, VL], dt.float32, tag="m")
                    nc.tensor.matmul(ai_ps[:], QGs[l][:, g:g + 1], HE[g][:],
                                     start=True, stop=True)
                    e = smp.tile([1, VL], bf, tag="s")
                    nc.scalar.activation(e[:], ai_ps[:], AF.Exp)
                    return e

                # software-pipelined: g0 core first, its epilogue DVE work
                # threaded between g1 core groups
                CGS = [(i * CG, CG) for i in range(UC // CG)]
                stwh_group(0)
                P_aggs = [ps_agg.tile([FP, VL], dt.float32, tag="agg",
                                      name=f"pagg{g}")
                          for g in range(2)]
                for k0, cg in CGS:
                    if k0 % SWG == 0 and k0 // SWG + 1 < NG:
                        stwh_group(k0 // SWG + 1)
                    core_group(0, k0, cg, P_aggs[0], 0)
                ncs0, den0 = epi_header(0, P_aggs[0])
                esplit = max(1, (2 * len(CGS)) // 3)
                for k0, cg in CGS[:esplit]:
                    core_group(1, k0, cg, P_aggs[1], 0)
                e0 = epi_tail(0, ncs0, den0)
                for k0, cg in CGS[esplit:]:
                    core_group(1, k0, cg, P_aggs[1], 0)
                ncs1, den1 = epi_header(1, P_aggs[1])
                e1 = epi_tail(1, ncs1, den1)

                # gated fuse
                ei = [e0, e1]
                dsum = smp.tile([1, VL], dt.float32, tag="s")
                nc.vector.tensor_tensor(out=dsum[:], in0=ei[0][:],
                                        in1=ei[1][:], op=op.add)
                rds = smp.tile([1, VL], dt.float32, tag="s")
                nc.vector.reciprocal(rds[:], dsum[:])
                b0 = smp.tile([1, VL], bf, tag="s")
                nc.vector.tensor_tensor(out=b0[:], in0=ei[0][:], in1=rds[:],
                                        op=op.mult)
                bib_ps = ps_m.tile([FP, VL], dt.float32, tag="m")
                nc.tensor.matmul(bib_ps[:], ONE1s[:], b0[:],
                                 start=True, stop=True)
                dd = work.tile([FP, VL], bf, tag="w")
                nc.vector.tensor_tensor(out=dd[:], in0=HE[0][:], in1=HE[1][:],
                                        op=op.subtract)
                bd = work.tile([FP, VL], bf, tag="w")
                nc.vector.tensor_tensor(out=bd[:], in0=dd[:], in1=bib_ps[:],
                                        op=op.mult)
                if hf_out is None:
                    return bd          # caller folds HE[1] + bd linearly
                nc.vector.tensor_tensor(out=hf_out[:], in0=HE[1][:],
                                        in1=bd[:], op=op.add)

            # ---------------- hop 1 ----------------
            gat_prologueA(0, XOWN)
            gat_main(0, xT, HF1)

            # all-gather H1 (feature-major, bf16); hop-2 own-slice prologue
            # runs under the collective (depends only on HF1).
            ag_in = drp.tile([FP, VL], bf)
            ag_out = drp.tile([NCORES, FP, VL], bf)
            nc.sync.dma_start(out=ag_in[:], in_=HF1[:])
            agv = ag_out.opt().rearrange("c (f v) -> c f v", v=VL)
            h1v = H1T.rearrange("f (c v) -> f c v", v=VL)
            if NO_COLLECTIVE:
                for c in range(NCORES):
                    nc.sync.dma_start(out=h1v[:, c, :], in_=ag_in[:])
            else:
                nc.gpsimd.collective_compute(
                    "AllGather", op.bypass,
                    replica_groups=[list(range(NCORES))],
                    ins=[ag_in.opt()], outs=[ag_out.opt()])
                for c in range(NCORES):
                    nc.sync.dma_start(out=h1v[:, c, :], in_=agv[c])
            gat_prologueA(1, HF1)

            # ---------------- hop 2 + MLP head ----------------
            # H2 = HE[1] + bd is folded into the first MLP matmul (linear)
            bd2 = gat_main(1, H1T, None)
            h_ps = ps_m.tile([MH, VL], dt.float32, tag="m")
            nc.tensor.matmul(h_ps[:], MW1[:], HE[1][:], start=True, stop=False)
            nc.tensor.matmul(h_ps[:], MW1[:], bd2[:], start=False, stop=True)
            hd = smp.tile([MH, VL], bf, tag="s")
            nc.vector.tensor_scalar(hd[:], h_ps[:], MB1[:], 0.0,
                                    op.add, op.max)
            o_ps = ps_m.tile([1, VL], dt.float32, tag="m")
            nc.tensor.matmul(o_ps[:], MW2[:], hd[:], start=True, stop=True)
            osb = smp.tile([1, VL], dt.float32, tag="s")
            nc.vector.tensor_scalar(osb[:], o_ps[:], MB2[:], None,
                                    op.add, op.bypass)
            nc.sync.dma_start(out=out_d.ap(), in_=osb[:])

    nc.compile()
    return nc


def _pad_rows(w):
    out = np.zeros((FP,) + w.shape[1:], dtype=np.float32)
    for h in range(HEADS):
        out[BLK * h:BLK * h + HD] = w[HD * h:HD * h + HD]
    return out


def _ahat(a):
    A = np.zeros((HID, 2 * HEADS), dtype=np.float32)
    for h in range(HEADS):
        A[HD * h:HD * h + HD, h] = a[h, :HD]
        A[HD * h:HD * h + HD, HEADS + h] = a[h, HD:]
    return A


def _prep_adj(adj, c):
    """(N,N) int -> per-core (P, UC*VL) bf16 {0,1} chunk layout of adjT."""
    sl = adj[c * VL:(c + 1) * VL, :].T.astype(np.float32)       # (N, VL)
    sl = sl.reshape(UC, P, VL).transpose(1, 0, 2).reshape(P, UC * VL)
    return np.ascontiguousarray(sl).astype(ml_dtypes.bfloat16)


def _bf(x):
    return np.ascontiguousarray(x).astype(ml_dtypes.bfloat16)


def kernel(**inputs):
    from concourse.bass_utils import run_bass_kernel_spmd

    if "nc" not in _CACHE:
        _CACHE["nc"] = _build()
    nc = _CACHE["nc"]

    f32 = np.float32
    x = np.asarray(inputs["x"], f32)
    adj = [np.asarray(inputs["adj_ind"]), np.asarray(inputs["adj_cor"])]
    W1 = [np.asarray(inputs["W1i"], f32), np.asarray(inputs["W1c"], f32)]
    W2 = [np.asarray(inputs["W2i"], f32), np.asarray(inputs["W2c"], f32)]
    A1 = [np.asarray(inputs["a1i"], f32), np.asarray(inputs["a1c"], f32)]
    A2 = [np.asarray(inputs["a2i"], f32), np.asarray(inputs["a2c"], f32)]
    q1 = [np.asarray(inputs["q1i"], f32), np.asarray(inputs["q1c"], f32)]
    q2 = [np.asarray(inputs["q2i"], f32), np.asarray(inputs["q2c"], f32)]

    common = {"xT": _bf(x.T)}
    for l, (Ws, As) in enumerate(((W1, A1), (W2, A2))):
        blocks = []
        for g in range(2):
            W = Ws[g] if l == 0 else _pad_rows(Ws[g])
            WA = W @ _ahat(As[g])
            blocks.append(np.hstack([W, WA]))
        common[f"WST{l}"] = _bf(np.hstack(blocks))
    for l, qs in enumerate((q1, q2)):
        common[f"qg{l}"] = _bf(
            np.stack([_pad_rows(qs[0][:, None])[:, 0],
                      _pad_rows(qs[1][:, None])[:, 0]], axis=1))
    common["mw1"] = _bf(_pad_rows(np.asarray(inputs["mlp_w1"], f32)))
    common["mb1"] = np.ascontiguousarray(
        np.asarray(inputs["mlp_b1"], f32)[:, None])
    common["mw2"] = _bf(np.asarray(inputs["mlp_w2"], f32))
    common["mb2"] = np.asarray(inputs["mlp_b2"], f32).reshape(1, 1)

    in_maps = []
    for c in range(NCORES):
        m = dict(common)
        m["xOwnT"] = _bf(x[c * VL:(c + 1) * VL, :].T)
        m["adjTB_i"] = _prep_adj(adj[0], c)
        m["adjTB_c"] = _prep_adj(adj[1], c)
        in_maps.append(m)

    res = run_bass_kernel_spmd(nc, in_maps, core_ids=list(range(NCORES)))
    out = np.concatenate([r["out"][0] for r in res.results])[:, None]
    return out.astype(np.float32)


if __name__ == "__main__":
    _CACHE["nc"] = _build()
    print("build ok")
